# revision 11
# baseline (speedup 1.0000x reference)
"""CascadedGroupCrossAttention1D Trainium2 kernel.

Data-parallel over batch: 8 cores x 4 batch items. Each core runs the full
8-head cascaded cross-attention on its batch shard.

Host-side preprocessing folds all BatchNorms into weights/biases:
  - pw-BN scale folded into qkv weights; q-bias via conv-pad trick; k-bias
    added at drain; v-bias (t_v) deferred: folded into next head's input chunk
    (host) and into the relu bias (device).
  - dw-BN folded into conv taps (diag matmuls on PE) + bias matmul.
  - attn softmax: scores in [m, l] layout, exp without max subtraction
    (logits are small; validated vs reference), attn_bias folded via an
    augmented k row, denominator S via an all-ones column in v^T, reciprocal
    on DVE + gpsimd partition_broadcast.
  - proj-BN folded into proj weights; output computed directly in [l, o]
    layout so no output transpose is needed.

Device layout per head, per group of 4 instances (instance = (batch', path)):
  phase 1 (psum "big" tile, 4 banks):
    q matmuls (M=16, col pos 32i) / k matmuls (M=32 zero-padded) /
    v^T matmuls ([64,128] lhsT from feat) ; drains to SBUF; conv via 5 diag
    matmuls + bias matmul (writes the q-hat ones row used to pick up the
    k-side bias row in the K=32 QK matmul).
  phase 2: per instance: QK (K=32) -> pscore [128, 2048]; one ACT exp;
    AV (4 accumulating matmuls, M=65 with S in row 64) into pav [65, 2048].
  epilogue: batched reciprocal of S rows, gpsimd broadcast, normalize (DVE),
    relu into relu_cat (gpsimd), feat update (gpsimd) with next input chunk.
Final: proj matmuls (relu_cat as lhsT) -> [l, o] output tiles -> DMA.
"""
import numpy as np
from contextlib import ExitStack

import concourse.bass as bass
import concourse.mybir as mybir
import concourse.tile as tile
from concourse import bacc
from concourse.bass_utils import run_bass_kernel_spmd

F32 = mybir.dt.float32
AL = mybir.AluOpType
AF = mybir.ActivationFunctionType

EPS = 1e-5
B, L, C, H = 32, 512, 512, 8
KD, D, KSZ = 16, 64, 5
CIN = C // H          # 64
N_CORES = 8
BC = B // N_CORES     # 4 batch per core
SCALE = KD ** -0.5    # 0.25

_CACHE = {}


# --------------------------------------------------------------------------
# host-side parameter preprocessing
# --------------------------------------------------------------------------
def _bn_fold(g, b, m, v):
    s = (g / np.sqrt(v + EPS)).astype(np.float32)
    t = (b - m * s).astype(np.float32)
    return s, t


def _prep_consts(inp):
    """Build all device const tensors in exact [partition, free] layouts."""
    out = {}
    paths = []
    for sfx in ("x", "y"):
        p = {k[:-2]: inp[k] for k in inp if k.endswith("_" + sfx) and k not in ("x", "y")}
        s_qkv, t_qkv = _bn_fold(p["qkv_g"], p["qkv_b"], p["qkv_m"], p["qkv_v"])  # [H,96]
        Wf = (p["qkv_w"] * s_qkv[:, :, None]).astype(np.float32)                # [H,96,64]
        s_dw, t_dw = _bn_fold(p["dw_g"], p["dw_b"], p["dw_m"], p["dw_v"])       # [H,16]
        s_p, t_p = _bn_fold(p["proj_g"], p["proj_b"], p["proj_m"], p["proj_v"])  # [C]
        paths.append(dict(
            WqT=np.transpose(Wf[:, :KD, :], (0, 2, 1)),          # [H,64,16]
            WkT=np.transpose(Wf[:, KD:2 * KD, :], (0, 2, 1)),    # [H,64,16]
            WvT=np.transpose(Wf[:, 2 * KD:, :], (0, 2, 1)),      # [H,64,64] (s_v folded)
            t_q=t_qkv[:, :KD], t_k=t_qkv[:, KD:2 * KD], t_v=t_qkv[:, 2 * KD:],
            tapw=(p["dw_w"] * s_dw[:, :, None]).astype(np.float32),  # [H,16,5]
            b_q=(s_dw * t_qkv[:, :KD] * p["dw_w"].sum(-1) + t_dw).astype(np.float32),
            WpT=np.ascontiguousarray((p["proj_w"] * s_p[:, None]).T.astype(np.float32)),  # [512(c),512(o)]
            t_p=t_p,
        ))

    # wq_sb [128, 8*32]: rows 64*path, cols 32h (cols 16:32 zero -> M=32 writes
    # the full 32-row block so no psum row is left uninitialized)
    wq = np.zeros((128, 256), np.float32)
    wk = np.zeros((128, 256), np.float32)   # cols 32h: 0:16 = WkT, 16:32 = 0
    wv = np.zeros((128, 512), np.float32)
    for pi, pp in enumerate(paths):
        r = 64 * pi
        for h in range(H):
            wq[r:r + 64, 32 * h:32 * h + 16] = pp["WqT"][h]
            wk[r:r + 64, 32 * h:32 * h + 16] = pp["WkT"][h]
            wv[r:r + 64, 64 * h:64 * h + 64] = pp["WvT"][h]
    out["wq"] = wq
    out["wk"] = wk
    out["wv"] = wv

    # dq [128, 8*5*128]: diag conv taps (per head, per tap); rows/cols 32i+r
    dq = np.zeros((128, H * KSZ * 128), np.float32)
    for h in range(H):
        for j in range(KSZ):
            base = (h * KSZ + j) * 128
            for i in range(4):
                for r in range(KD):
                    # tap weight identical for x/y? NO: tapw differs per path!
                    pass
    # conv taps: block i holds q of the PARTNER path (cross attention):
    # instance i consumes q from (same batch, other path) -> path (i+1) % 2
    for h in range(H):
        for j in range(KSZ):
            base = (h * KSZ + j) * 128
            for i in range(4):
                pp = paths[(i + 1) % 2]
                for r in range(KD):
                    dq[32 * i + r, base + 32 * i + r] = pp["tapw"][h, r, j]
    out["dq"] = dq

    # bq [1, 8*128]: conv bias column; rows 32i+r = b_q[path]; row 32i+16 = 1.0
    bq = np.zeros((1, H * 128), np.float32)
    for h in range(H):
        for i in range(4):
            pp = paths[(i + 1) % 2]
            bq[0, 128 * h + 32 * i:128 * h + 32 * i + KD] = pp["b_q"][h]
            bq[0, 128 * h + 32 * i + 16] = 1.0
    out["bq"] = bq

    # tk [128, 8]: k-drain bias rows
    tk = np.zeros((128, H), np.float32)
    for h in range(H):
        for i in range(4):
            tk[32 * i:32 * i + KD, h] = paths[i % 2]["t_k"][h]
    out["tk"] = tk

    # padc [128, 8*4]: -t_q pads (cols 4h+{0,1,2,3})
    padc = np.zeros((128, H * 4), np.float32)
    for h in range(H):
        for i in range(4):
            padc[32 * i:32 * i + KD, 4 * h:4 * h + 4] = -paths[(i + 1) % 2]["t_q"][h][:, None]
    out["padc"] = padc

    # kbias [1, 8*512]: 4*attn_bias per head (pre-divided by exp scale)
    kb = (inp["attn_bias"].astype(np.float32) / SCALE).reshape(1, H * L)
    out["kbias"] = np.ascontiguousarray(kb)

    # e16 [1, 128]: ones at rows 32i+16 (lhsT for kbias matmul)
    e16 = np.zeros((1, 128), np.float32)
    e16[0, 16::32] = 1.0
    out["e16"] = e16

    # tv [64, 16]: relu bias, col 2h+path
    tv = np.zeros((64, 2 * H), np.float32)
    for h in range(H):
        for pi in range(2):
            tv[:, 2 * h + pi] = paths[pi]["t_v"][h]
    out["tv"] = tv

    # wpt [128, 2*4*512]: proj weight chunks, col block (4*path+ci)*512
    wpt = np.zeros((128, 2 * 4 * 512), np.float32)
    for pi, pp in enumerate(paths):
        for ci in range(4):
            wpt[:, (4 * pi + ci) * 512:(4 * pi + ci + 1) * 512] = pp["WpT"][128 * ci:128 * ci + 128, :]
    out["wpt"] = wpt

    # tp [1, 2*512]
    tp = np.zeros((1, 2 * 512), np.float32)
    tp[0, :512] = paths[0]["t_p"]
    tp[0, 512:] = paths[1]["t_p"]
    out["tp"] = tp

    # t_v host-fold for input chunks
    out["_tv_x"] = paths[0]["t_v"]
    out["_tv_y"] = paths[1]["t_v"]
    return out


def _prep_inputs_core(x_c, y_c, tv_x, tv_y):
    """[4, L, C] -> biased transposed [4, C, L]."""
    xt = np.ascontiguousarray(np.transpose(x_c, (0, 2, 1))).astype(np.float32)
    yt = np.ascontiguousarray(np.transpose(y_c, (0, 2, 1))).astype(np.float32)
    for h in range(1, H):
        xt[:, CIN * h:CIN * (h + 1), :] += tv_x[h - 1][None, :, None]
        yt[:, CIN * h:CIN * (h + 1), :] += tv_y[h - 1][None, :, None]
    return xt, yt


# --------------------------------------------------------------------------
# device kernel
# --------------------------------------------------------------------------
def _build_bass(dbg=False):
    nc = bacc.Bacc("TRN2", target_bir_lowering=False, debug=False, num_devices=N_CORES)

    xt_d = nc.declare_dram_parameter("xt", [BC, C, L], F32, False)
    yt_d = nc.declare_dram_parameter("yt", [BC, C, L], F32, False)
    wq_d = nc.declare_dram_parameter("wq", [128, 256], F32, False)
    wk_d = nc.declare_dram_parameter("wk", [128, 256], F32, False)
    wv_d = nc.declare_dram_parameter("wv", [128, 512], F32, False)
    dq_d = nc.declare_dram_parameter("dq", [128, H * KSZ * 128], F32, False)
    bq_d = nc.declare_dram_parameter("bq", [1, H * 128], F32, False)
    tk_d = nc.declare_dram_parameter("tk", [128, H], F32, False)
    padc_d = nc.declare_dram_parameter("padc", [128, H * 4], F32, False)
    kbias_d = nc.declare_dram_parameter("kbias", [1, H * L], F32, False)
    e16_d = nc.declare_dram_parameter("e16", [1, 128], F32, False)
    tv_d = nc.declare_dram_parameter("tv", [64, 2 * H], F32, False)
    wpt_d = nc.declare_dram_parameter("wpt", [128, 2 * 4 * 512], F32, False)
    tp_d = nc.declare_dram_parameter("tp", [1, 2 * 512], F32, False)
    ox_d = nc.declare_dram_parameter("out_x", [BC, L, C], F32, True)
    oy_d = nc.declare_dram_parameter("out_y", [BC, L, C], F32, True)
    if dbg:
        dbg_d = {n: nc.declare_dram_parameter(n, sh, F32, True) for n, sh in [
            ("dbg_qhat", [128, 512]), ("dbg_khat", [128, 512]),
            ("dbg_vtsb", [128, 2048]), ("dbg_e", [128, 2048]),
            ("dbg_pav", [128, 2048]), ("dbg_ton", [64, 512]),
            ("dbg_feat0", [128, 512]), ("dbg_tileA", [128, 516])]}

    with tile.TileContext(nc) as tc, ExitStack() as ctx:
        cst = ctx.enter_context(tc.tile_pool(name="cst", bufs=1))
        per = ctx.enter_context(tc.tile_pool(name="per", bufs=1))
        wrk = ctx.enter_context(tc.tile_pool(name="wrk", bufs=2))
        xp = ctx.enter_context(tc.tile_pool(name="xp", bufs=3))
        ps = ctx.enter_context(tc.tile_pool(name="ps", bufs=1, space="PSUM"))

        # ---- consts to SBUF
        def cload(dram, shape, tag):
            t = cst.tile(shape, F32, tag=tag)
            nc.gpsimd.dma_start(t[:], dram[:])
            return t

        wq = cload(wq_d, [128, 256], "wq")
        wk = cload(wk_d, [128, 256], "wk")
        wv = cload(wv_d, [128, 512], "wv")
        bq = cload(bq_d, [1, H * 128], "bq")
        tk = cload(tk_d, [128, H], "tk")
        padc = cload(padc_d, [128, H * 4], "padc")
        kbias = cload(kbias_d, [1, H * L], "kbias")
        e16 = cload(e16_d, [1, 128], "e16")
        tv = cload(tv_d, [64, 2 * H], "tv")
        tp = cload(tp_d, [1, 2 * 512], "tp")
        onesrow = cst.tile([1, 512], F32, tag="onesrow")
        nc.gpsimd.memset(onesrow[:], 1.0)
        onescol = cst.tile([1, 128], F32, tag="onescol")
        nc.gpsimd.memset(onescol[:], 1.0)

        # ---- persistent tiles
        feat = [per.tile([128, 512], F32, tag=f"feat{b}", name=f"feat{b}")
                for b in range(BC)]
        relu_cat = [[per.tile([128, 2048], F32, tag=f"relu{b}_{p}", name=f"relu{b}_{p}")
                     for p in range(2)]
                    for b in range(BC)]
        for b in range(BC):
            nc.sync.dma_start(feat[b][0:64, :], xt_d[b, 0:CIN, :])
            nc.sync.dma_start(feat[b][64:128, :], yt_d[b, 0:CIN, :])

        def mm(out, lhsT, rhs, start, stop, tpos, skip=False):
            nc.tensor.matmul(out, lhsT, rhs, start=start, stop=stop,
                             tile_position=tpos, skip_group_check=skip)

        # ---- head cascade
        for h in range(H):
            for g in range(2):
                psA = ps.tile([128, 2048], F32, tag="big")
                # phase 1 matmuls: q (M=16), k (M=32 padded), vT
                for i in range(4):
                    b = 2 * g + i // 2
                    p = i % 2
                    r = 64 * p
                    rq = 64 * ((i + 1) % 2)   # q comes from the partner path
                    mm(psA[32 * i:32 * i + 32, 0:512],
                       wq[rq:rq + 64, 32 * h:32 * h + 32], feat[b][rq:rq + 64, :],
                       True, True, (rq, 32 * i))
                    mm(psA[32 * i:32 * i + 32, 512:1024],
                       wk[r:r + 64, 32 * h:32 * h + 32], feat[b][r:r + 64, :],
                       True, True, (r, 32 * i))
                    for j in range(4):
                        mm(psA[:, 1024 + 256 * i + 64 * j:1024 + 256 * i + 64 * j + 64],
                           feat[b][r:r + 64, 128 * j:128 * j + 128],
                           wv[r:r + 64, 64 * h:64 * h + 64],
                           True, True, (r, 0))
                # kbias row matmul (accumulates into k region)
                mm(psA[:, 512:1024], e16[:], kbias[0:1, L * h:L * (h + 1)],
                   False, True, (0, 0), skip=True)

                # drains
                tileA = wrk.tile([128, 516], F32, tag="tileA", bufs=1)
                nc.vector.tensor_scalar(tileA[:, 2:514], psA[:, 0:512], 0.0, None, AL.add)
                nc.vector.tensor_scalar(tileA[:, 0:2], padc[:, 4 * h:4 * h + 2], 0.0, None, AL.add)
                nc.vector.tensor_scalar(tileA[:, 514:516], padc[:, 4 * h + 2:4 * h + 4], 0.0, None, AL.add)
                khat = wrk.tile([128, 512], F32, tag="khat")
                nc.vector.tensor_scalar(khat[:], psA[:, 512:1024], tk[:, h:h + 1], None, AL.add)
                vtsb = wrk.tile([128, 2048], F32, tag="vtsb")
                # vtsb block k (128 cols): col0 = ones -> S in pav row 0;
                # cols 1:64 = zeros (pav rows 1:63 = junk, never read);
                # cols 64:128 = vT chunk -> out_u in pav rows 64:127 (aligned).
                nc.gpsimd.memset(vtsb[:].rearrange("p (c e) -> p c e", e=128)[:, :, 0:1], 1.0)
                nc.gpsimd.memset(vtsb[:].rearrange("p (c e) -> p c e", e=128)[:, :, 1:64], 0.0)
                nc.vector.tensor_scalar(
                    vtsb[:].rearrange("p (c e) -> p c e", e=128)[:, :, 64:128],
                    psA[:, 1024:2048].rearrange("p (c e) -> p c e", e=64),
                    0.0, None, AL.add)

                # conv: separate psum tile (score tag is free during phase 1)
                dqr = wrk.tile([128, KSZ * 128], F32, tag="dqr")
                nc.sync.dma_start(dqr[:], dq_d[:, h * KSZ * 128:(h + 1) * KSZ * 128])
                pconv = ps.tile([128, 2048], F32, tag="score", name="pconv")
                for j in range(KSZ):
                    mm(pconv[:, 0:512], dqr[:, 128 * j:128 * j + 128],
                       tileA[:, j:j + 512], j == 0, False, (0, 0))
                mm(pconv[:, 0:512], bq[0:1, 128 * h:128 * h + 128], onesrow[:],
                   False, True, (0, 0))
                qhat = wrk.tile([128, 512], F32, tag="qhat")
                nc.vector.tensor_scalar(qhat[:], pconv[:, 0:512], 0.0, None, AL.add)
                if dbg and h == 0 and g == 0:
                    nc.sync.dma_start(dbg_d["dbg_qhat"][:], qhat[:])
                    nc.sync.dma_start(dbg_d["dbg_khat"][:], khat[:])
                    nc.sync.dma_start(dbg_d["dbg_vtsb"][:], vtsb[:])
                    nc.sync.dma_start(dbg_d["dbg_tileA"][:], tileA[:])

                # phase 2: per-instance QK / exp / AV
                pav = ps.tile([128, 2048], F32, tag="big")
                esbs = []
                for i in range(4):
                    pscore = ps.tile([128, 2048], F32, tag="score")
                    for j in range(4):
                        mm(pscore[:, 512 * j:512 * j + 512],
                           khat[32 * i:32 * i + 32, 128 * j:128 * j + 128],
                           qhat[32 * i:32 * i + 32, :],
                           True, True, (32 * i, 0))
                    e_sb = wrk.tile([128, 2048], F32, tag="esb")
                    nc.scalar.activation(e_sb[:], pscore[:], AF.Exp, scale=SCALE)
                    esbs.append(e_sb)
                    if dbg and h == 0 and g == 0 and i == 0:
                        nc.sync.dma_start(dbg_d["dbg_e"][:], e_sb[:])
                    for j in range(4):
                        mm(pav[:, 512 * i:512 * i + 512],
                           vtsb[:, 128 * (4 * i + j):128 * (4 * i + j) + 128],
                           e_sb[:, 512 * j:512 * j + 512],
                           j == 0, j == 3, (0, 0))

                # epilogue
                r4 = wrk.tile([1, 2048], F32, tag="r4", bufs=1)
                if dbg and h == 0 and g == 0:
                    pavsb = wrk.tile([128, 2048], F32, tag="esb", name="pavsb")
                    nc.vector.tensor_scalar(pavsb[:], pav[:], 0.0, None, AL.add)
                    nc.sync.dma_start(dbg_d["dbg_pav"][:], pavsb[:])
                nc.vector.reciprocal_approx_fast(r4[:], pav[0:1, :])
                for i in range(4):
                    b = 2 * g + i // 2
                    p = i % 2
                    rB = wrk.tile([64, 512], F32, tag="rB")
                    nc.gpsimd.partition_broadcast(rB[:], r4[0:1, 512 * i:512 * i + 512])
                    t_on = wrk.tile([64, 512], F32, tag="t_on")
                    nc.vector.scalar_tensor_tensor(
                        t_on[:], pav[64:128, 512 * i:512 * i + 512], 1.0, rB[:],
                        AL.mult, AL.mult)
                    nc.gpsimd.tensor_scalar(
                        relu_cat[b][p][64 * (h % 2):64 * (h % 2) + 64,
                                       512 * (h // 2):512 * (h // 2) + 512],
                        t_on[:], tv[:, 2 * h + p:2 * h + p + 1], 0.0, AL.add, AL.max)
                    if dbg and h == 0 and g == 0 and i == 0:
                        nc.sync.dma_start(dbg_d["dbg_ton"][:], t_on[:])
                    if h < H - 1:
                        xch = xp.tile([64, 512], F32, tag="xch")
                        src_d = xt_d if p == 0 else yt_d
                        nc.sync.dma_start(xch[:], src_d[b, CIN * (h + 1):CIN * (h + 2), :])
                        nc.gpsimd.tensor_tensor(feat[b][64 * p:64 * p + 64, :],
                                                t_on[:], xch[:], AL.add)

        if dbg:
            nc.sync.dma_start(dbg_d["dbg_feat0"][:], feat[0][:])
        # ---- proj phase
        for p in range(2):
            wptt = wrk.tile([128, 2048], F32, tag="esb", name=f"wptt{p}")
            nc.sync.dma_start(wptt[:], wpt_d[:, 4 * p * 512:(4 * p + 4) * 512])
            o_d = ox_d if p == 0 else oy_d
            for b in range(BC):
                for lj in range(4):
                    pproj = ps.tile([128, 512], F32, tag="score")
                    for ci in range(4):
                        mm(pproj[:], relu_cat[b][p][:, 512 * ci + 128 * lj:512 * ci + 128 * lj + 128],
                           wptt[:, 512 * ci:512 * ci + 512],
                           ci == 0, False, (0, 0))
                    mm(pproj[:], onescol[:], tp[0:1, 512 * p:512 * p + 512],
                       False, True, (0, 0))
                    osb = wrk.tile([128, 512], F32, tag="osb")
                    nc.vector.tensor_scalar(osb[:], pproj[:], 0.0, None, AL.add)
                    nc.sync.dma_start(o_d[b, 128 * lj:128 * lj + 128, :], osb[:])

    nc.compile()
    return nc


# --------------------------------------------------------------------------
# entry point
# --------------------------------------------------------------------------
def kernel(**inputs):
    consts = _prep_consts(inputs)
    tv_x = consts.pop("_tv_x")
    tv_y = consts.pop("_tv_y")

    if "nc" not in _CACHE:
        _CACHE["nc"] = _build_bass()
    nc = _CACHE["nc"]

    x = np.asarray(inputs["x"], np.float32)
    y = np.asarray(inputs["y"], np.float32)
    in_maps = []
    for c in range(N_CORES):
        xt, yt = _prep_inputs_core(x[BC * c:BC * (c + 1)], y[BC * c:BC * (c + 1)],
                                   tv_x, tv_y)
        m = {"xt": xt, "yt": yt}
        m.update(consts)
        in_maps.append(m)

    res = run_bass_kernel_spmd(nc, in_maps, list(range(N_CORES))).results
    rx = np.concatenate([res[c]["out_x"] for c in range(N_CORES)], axis=0)
    ry = np.concatenate([res[c]["out_y"] for c in range(N_CORES)], axis=0)
    return rx, ry
